# revision 16
# baseline (speedup 1.0000x reference)
"""MixLinear int4-GEMM kernel for 8x TRN2 NeuronCores.

Strategy: tensor-parallel over out_features (each core owns OUT/8 = 512
output channels; q_weight / scale_col / weight_cache are sharded along the
output dim; x is replicated).  Per core:

  1. Per 128-row activation tile: masked abs-max (outlier cols excluded) in
     one fused DVE tensor_tensor_reduce pass; x_scale = max/7, r = 1/x_scale.
  2. Magic-number RNE round: t = xz*r + 1.5*2^23 on ScalarE, q = t - magic on
     GPSIMD (exact small ints, cast to bf16).
  3. q is transposed to contraction-major layout with one DMA-xbar transpose.
  4. int4 weights are unpacked on-device (DVE bitwise ops on the packed
     bytes) into a [K, 32, 512] bf16 wT resident in SBUF, transposed by
     DMA-xbar.
  5. 32 bf16 matmuls (exact: integer values) + 2 outlier matmuls accumulate
     into one PSUM bank.  The outlier operands are pre-scaled by 1/x_scale
     (per row) and 1/scale_col (per out channel) so one dequant applies to
     the whole PSUM: y = psum * x_scale * scale_col, fused into the PSUM
     eviction (ScalarE per-partition scale, DVE broadcast multiply).

The output shard [8192, 512] is DMA'd out; the host concatenates shards.
"""

import numpy as np

B, S, IN, OUT, FP = 4, 2048, 4096, 4096, 256
M = B * S
NCORES = 8
OS = OUT // NCORES  # out-features shard per core
QMAX = 7.0
MAGIC = 12582912.0  # 1.5 * 2**23: adding+subtracting forces RNE to integer


def emit_core_kernel(nc, tc, m, in_dim, os_dim, fp_dim):
    """Emit the per-core tile program. All dims compile-time constants."""
    import os as _os

    import concourse.bass as bass
    import concourse.mybir as mybir
    from concourse.masks import make_identity

    DBG = set(_os.environ.get("KERNEL_DISABLE", "").split(","))

    f32 = mybir.dt.float32
    f32r = mybir.dt.float32r
    bf16 = mybir.dt.bfloat16
    i32 = mybir.dt.int32
    i16 = mybir.dt.int16
    Alu = mybir.AluOpType
    Act = mybir.ActivationFunctionType

    P = 128
    MT = m // P              # number of 128-row activation tiles
    KT = in_dim // P         # number of 128-deep contraction tiles
    FT = fp_dim // P         # outlier contraction tiles (2)
    OJ = os_dim // P         # out-shard subtiles (4)

    x = nc.dram_tensor("x", [m, in_dim], f32, kind="ExternalInput")
    qw = nc.dram_tensor("qw", [os_dim, in_dim // 2], i32, kind="ExternalInput")
    sc = nc.dram_tensor("sc", [os_dim], f32, kind="ExternalInput")
    wc = nc.dram_tensor("wc", [os_dim, fp_dim], f32, kind="ExternalInput")
    maskrow = nc.dram_tensor("maskrow", [in_dim], f32, kind="ExternalInput")
    idx = nc.dram_tensor("idx", [P, fp_dim // 16], i16, kind="ExternalInput")
    y = nc.dram_tensor("y", [m, os_dim], f32, kind="ExternalOutput")

    with (
        tc.tile_pool(name="const", bufs=1) as const,
        tc.tile_pool(name="wstage", bufs=1) as wstage,
        tc.tile_pool(name="xp", bufs=2) as xp,
        tc.tile_pool(name="xzp", bufs=2) as xzp,
        tc.tile_pool(name="qp", bufs=2) as qp,
        tc.tile_pool(name="qtp", bufs=2) as qtp,
        tc.tile_pool(name="aop", bufs=2) as aop,
        tc.tile_pool(name="aotp", bufs=2) as aotp,
        tc.tile_pool(name="sp", bufs=6) as sp,
        tc.tile_pool(name="yp", bufs=2) as yp,
        tc.tile_pool(name="py", bufs=2, space="PSUM") as py,
        tc.tile_pool(name="ptp", bufs=2, space="PSUM") as ptp,
    ):
        # ---------------- one-time setup ----------------
        from concourse import library_config

        if "gather" not in DBG:
            nc.gpsimd.load_library(library_config.ap_gather)

        identity = const.tile([P, P], f32)
        make_identity(nc, identity[:])

        # outlier mask broadcast to all partitions: maskF[p, k] = 0 iff k in ind
        maskF = const.tile([P, in_dim], f32)
        nc.sync.dma_start(maskF[:], maskrow[None, :].to_broadcast((P, in_dim)))

        # wrapped gather indices for ap_gather
        idxs = const.tile([P, fp_dim // 16], i16)
        nc.sync.dma_start(idxs[:], idx[:])

        # scale_col shard: broadcast along partitions [P, OS] for dequant
        sc_bcast = const.tile([P, os_dim], f32)
        nc.sync.dma_start(sc_bcast[:], sc[None, :].to_broadcast((P, os_dim)))

        # scale_col per-partition view [P, OJ] for pre-dividing weight_cache
        sc_op = const.tile([P, OJ], f32)
        nc.sync.dma_start(sc_op[:], sc.rearrange("(j p) -> p j", p=P))

        # weight_cache': wc[o, f] / sc[o], transposed to [P_f, FT, OS] bf16
        wc_sb = wstage.tile([P, OJ, fp_dim], f32)
        nc.sync.dma_start(wc_sb[:], wc.rearrange("(j p) f -> p j f", p=P))
        rsc_op = const.tile([P, OJ], f32)
        nc.vector.reciprocal(rsc_op[:], sc_op[:])
        wcp = wstage.tile([P, OJ, fp_dim], f32)
        for j in range(OJ):
            nc.vector.tensor_scalar(
                wcp[:, j, :], wc_sb[:, j, :], rsc_op[:, j : j + 1], None, Alu.mult
            )
        wcT = const.tile([P, FT, os_dim], f32r)
        for j in range(OJ):
            for ff in range(FT):
                ps = ptp.tile([P, P], f32, tag="tp")
                nc.tensor.transpose(ps[:], wcp[:, j, ff * P : (ff + 1) * P], identity[:])
                nc.scalar.activation(
                    wcT[:, ff, j * P : (j + 1) * P], ps[:], Act.Copy
                )

        # int4 weight unpack: qw[o, i] byte -> w[o, 2i] = lo nibble signed,
        # w[o, 2i+1] = hi nibble signed; then DMA-xbar into wT [P_k, KT, OS]
        wT = const.tile([P, KT, os_dim], bf16)
        qw_v = qw.rearrange("(j p) k -> p j k", p=P)
        for j in range(OJ):
            qwj = wstage.tile([P, in_dim // 2], i32, tag="qwj")
            nc.sync.dma_start(qwj[:], qw_v[:, j, :])
            w_ok = wstage.tile([P, in_dim], bf16, tag="wok")
            w_ok_v = w_ok.rearrange("p (k two) -> p k two", two=2)
            tmp = wstage.tile([P, in_dim // 2], i32, tag="wtmp")
            # low nibble: ((v & 15) ^ 8) - 8
            nc.vector.tensor_scalar(
                tmp[:], qwj[:], 15, 8, Alu.bitwise_and, Alu.bitwise_xor
            )
            nc.vector.tensor_scalar(w_ok_v[:, :, 0], tmp[:], 8, None, Alu.subtract)
            # high nibble: (((v >> 4) & 15) ^ 8) - 8
            tmp2 = wstage.tile([P, in_dim // 2], i32, tag="wtmp2")
            nc.vector.tensor_scalar(
                tmp2[:], qwj[:], 4, None, Alu.arith_shift_right
            )
            nc.vector.tensor_scalar(
                tmp[:], tmp2[:], 15, 8, Alu.bitwise_and, Alu.bitwise_xor
            )
            nc.vector.tensor_scalar(w_ok_v[:, :, 1], tmp[:], 8, None, Alu.subtract)
            # transpose [128 o, in_dim k] -> wT[p_k, KT, o-chunk j]
            nc.sync.dma_start_transpose(wT[:, :, j * P : (j + 1) * P], w_ok[:])

        # ---------------- main loop over 128-row activation tiles ----------
        for mi in range(MT):
            x_t = xp.tile([P, in_dim], f32)
            nc.sync.dma_start(x_t[:], x[mi * P : (mi + 1) * P, :])

            # masked abs-max -> mx; xz = x * mask (outlier cols zeroed)
            xz = xzp.tile([P, in_dim], f32)
            mx = sp.tile([P, 1], f32, tag="mx")
            nc.vector.tensor_tensor(xz[:], x_t[:], maskF[:], Alu.mult)
            nc.vector.tensor_reduce(
                mx[:], xz[:], mybir.AxisListType.X, Alu.max,
                apply_absolute_value=True,
            )
            s_t = sp.tile([P, 1], f32, tag="s")
            nc.vector.tensor_scalar(s_t[:], mx[:], float(np.float32(1.0) / np.float32(QMAX)), None, Alu.mult)
            r_t = sp.tile([P, 1], f32, tag="r")
            nc.vector.reciprocal(r_t[:], s_t[:])

            # outlier activations: gather + pre-scale by r, transpose via PE
            ao = aop.tile([P, fp_dim], f32, tag="ao")
            if "gather" in DBG:
                nc.vector.tensor_copy(ao[:], x_t[:, :fp_dim])
            else:
                nc.gpsimd.ap_gather(
                    ao[:, :, None],
                    x_t[:, :, None],
                    idxs[:],
                    channels=P,
                    num_elems=in_dim,
                    d=1,
                    num_idxs=fp_dim,
                )
            aos = aop.tile([P, fp_dim], f32, tag="aos")
            nc.vector.tensor_scalar(aos[:], ao[:], r_t[:], None, Alu.mult)
            aoT = aotp.tile([P, FT, P], f32r)
            for ff in range(FT):
                ps = ptp.tile([P, P], f32, tag="tp")
                nc.tensor.transpose(ps[:], aos[:, ff * P : (ff + 1) * P], identity[:])
                nc.scalar.activation(aoT[:, ff, :], ps[:], Act.Copy)

            # quantize: t = xz * r + MAGIC (ScalarE), q = t - MAGIC (GPSIMD)
            nc.scalar.activation(
                x_t[:], xz[:], Act.Copy, bias=MAGIC, scale=r_t[:]
            )
            q = qp.tile([P, in_dim], bf16)
            if "gpsimdq" in DBG:
                nc.vector.tensor_scalar(q[:], x_t[:], -MAGIC, None, Alu.add)
            else:
                nc.gpsimd.tensor_scalar(q[:], x_t[:], -MAGIC, None, Alu.add)

            # transpose q to contraction-major via DMA xbar
            qT = qtp.tile([P, KT, P], bf16)
            nc.sync.dma_start_transpose(qT[:], q[:])

            # GEMMs: 32 int tiles + 2 outlier tiles accumulate in one bank
            psum = py.tile([P, os_dim], f32)
            for ko in range(KT):
                nc.tensor.matmul(
                    psum[:],
                    qT[:, ko, :],
                    wT[:, ko, :],
                    start=(ko == 0),
                    stop=False,
                )
            for ff in range(FT):
                nc.tensor.matmul(
                    psum[:],
                    aoT[:, ff, :],
                    wcT[:, ff, :],
                    start=False,
                    stop=(ff == FT - 1),
                )

            # dequant + store: y = psum * x_scale (ACT) * scale_col (DVE)
            t1 = yp.tile([P, os_dim], f32, tag="t1")
            nc.scalar.activation(t1[:], psum[:], Act.Copy, scale=s_t[:])
            yt = yp.tile([P, os_dim], f32, tag="yt")
            nc.vector.scalar_tensor_tensor(
                yt[:], t1[:], 1.0, sc_bcast[:], Alu.mult, Alu.mult
            )
            nc.sync.dma_start(y[mi * P : (mi + 1) * P, :], yt[:])

    return nc


def build_nc(m=M, in_dim=IN, os_dim=OS, fp_dim=FP):
    import concourse.bacc as bacc
    import concourse.tile as tile

    nc = bacc.Bacc(None, target_bir_lowering=False)
    with tile.TileContext(nc) as tc:
        emit_core_kernel(nc, tc, m, in_dim, os_dim, fp_dim)
    nc.compile()
    return nc


def make_host_inputs(x, q_weight, scale_col, weight_cache, ind,
                     m=M, in_dim=IN, os_dim=OS, fp_dim=FP, ncores=NCORES):
    """Shard/relayout full inputs into per-core input maps (no arithmetic)."""
    xf = np.ascontiguousarray(x.reshape(m, in_dim).astype(np.float32, copy=False))
    ind = np.asarray(ind).astype(np.int64)
    maskrow = np.ones(in_dim, dtype=np.float32)
    maskrow[ind] = 0.0
    w = ind.astype(np.int16).reshape(fp_dim // 16, 16)  # j = i*16 + (p%16)
    idx = np.tile(w.T, (8, 1)).astype(np.int16)  # [128, fp/16]
    scf = np.asarray(scale_col).reshape(-1).astype(np.float32, copy=False)

    in_maps = []
    for c in range(ncores):
        o0, o1 = c * os_dim, (c + 1) * os_dim
        in_maps.append(
            {
                "x": xf,
                "qw": np.ascontiguousarray(q_weight[o0:o1]).astype(np.int32, copy=False),
                "sc": np.ascontiguousarray(scf[o0:o1]),
                "wc": np.ascontiguousarray(weight_cache[o0:o1]).astype(np.float32, copy=False),
                "maskrow": maskrow,
                "idx": idx,
            }
        )
    return in_maps


_NC_CACHE = {}


def kernel(x, q_weight, scale_col, weight_cache, ind, trace=False):
    from concourse.bass_utils import run_bass_kernel_spmd

    key = "full"
    if key not in _NC_CACHE:
        _NC_CACHE[key] = build_nc()
    nc = _NC_CACHE[key]

    in_maps = make_host_inputs(x, q_weight, scale_col, weight_cache, ind)
    res = run_bass_kernel_spmd(nc, in_maps, list(range(NCORES)), trace=trace)
    yshards = [res.results[c]["y"] for c in range(NCORES)]
    yfull = np.concatenate(yshards, axis=1).reshape(B, S, OUT)
    if trace:
        return yfull, res
    return yfull
